# revision 60
# baseline (speedup 1.0000x reference)
"""Trainium2 Bass kernel for LogWignerCrystalSlaterFixedCYJastrow.

Computes, per walker (batch of 1024, 64 electrons in 3D, box L=20):
    out = logdet(Phi_up) + logdet(Phi_dn) + jastrow
where Phi_s are 32x32 Gaussian-orbital Slater matrices over 27 periodic
images (collapsed analytically to a separable per-axis 3-image sum), and
jastrow is a Coulomb-Yukawa pair sum with minimum-image wrapping.

Strategy: pure data parallel over 8 NeuronCores, 128 walkers per core,
one walker per SBUF partition.  The two 32x32 slogdets per walker are done
with a batched, in-SBUF Gaussian elimination with partial pivoting
(pivot row selected/extracted with indicator arithmetic -- no data
dependent control flow, identical instruction stream for all walkers).

v2 structure vs the earlier kernel:
 - Jastrow over a banded pair set (i, i+d mod 64), d=1..32 (2112 pairs/
   walker vs 4096 matrix entries): each unordered pair exactly once (the
   d=32 column weighted 0.5).  No diagonal masking needed.  Same/diff-spin
   exp selection folded into a single Exp via a static per-pair -1/F tile
   (GpSimd memsets at the idle head): e_sel = exp(-r/F_sel).
 - All jastrow transcendentals run on ScalarE inside the GE window; its
   elementwise tail is spread across GE steps as DVE fillers.  (The tail
   stays fp32: ScalarE activations writing bf16 outputs produced wrong
   values on this hardware, so bf16 is restricted to the GE matrix ops.)
 - Mixed precision GE: the first KBF=11 elimination steps run on a bf16
   matrix (pivot search candidates and pivot rows stay fp32), which gets
   2x DVE throughput on the packed extraction-mult and update-subtract;
   the transition step's subtract writes the trailing block directly in
   fp32.  The bf16 rank-1 outer product also reaches 2x via pair
   duplication: prs is written as adjacent pairs so every operand of the
   per-spin outer keeps a packed [stride-1 x 2] innermost dim (a plain
   broadcast has innermost stride 0, which disables the fast mode).  Measured max rel err 1.59e-2 on the fixed harness inputs (gate
   2e-2, fully deterministic); the fp32-only variant measures 1.25e-4 but
   is ~12us slower.
 - logdet read off the pivot-search maxima: Mb_k = piv_k^2 * w_ipiv and
   every row pivots exactly once, so ld = 0.5*(sum ln Mb_k - sum ln w_i),
   with the weight constant folded into the jastrow accumulation bias and
   the Ln table preloaded during the GE so the tail pays no swap.
"""

import os
import sys
import numpy as np
from contextlib import ExitStack

for _p in ("/opt/trn_rl_repo", "/opt/pypackages"):
    if _p not in sys.path:
        sys.path.append(_p)

import concourse.bass as bass
import concourse.bacc as bacc
import concourse.mybir as mybir
import concourse.tile as tile
from concourse.bass import AP
from concourse.bass_utils import run_bass_kernel_spmd

P = 128          # partitions = walkers per core
NCORES = 8
B = 1024
N = 64           # electrons per walker
NS = 32          # electrons / orbitals per spin
ND = 32          # banded jastrow offsets (d = 1..32)
L = 20.0
F32 = mybir.dt.float32
BF16 = mybir.dt.bfloat16
AF = mybir.ActivationFunctionType
OP = mybir.AluOpType
AX = mybir.AxisListType
KBF = 12         # GE steps 0..KBF-1 run on a bf16 matrix (2x DVE on packed ops)


def _centers():
    n = 1
    while n ** 3 < NS:
        n += 1
    a = L / n
    coords = np.linspace(0.0, L - a, n)
    grid = np.stack(np.meshgrid(coords, coords, coords, indexing="ij"), axis=-1)
    grid = grid.reshape(-1, 3)
    cu = grid[:NS].astype(np.float32)
    cd = (grid + a / 2)[:NS].astype(np.float32)
    return cu, cd


def _jastrow_consts():
    dens = np.float32(N / L ** 3)
    A = np.float32(1.0) / np.sqrt(np.float32(4 * np.pi) * dens, dtype=np.float32)
    Fs = np.sqrt(np.float32(2.0) * A, dtype=np.float32)
    Fd = np.sqrt(A, dtype=np.float32)
    return float(A), float(Fs), float(Fd)


def _build(alpha: float) -> bass.Bass:
    nc = bacc.Bacc()
    # xsh: electron coords doubled along the electron axis (96 = 64 + 32)
    xsh = nc.declare_dram_parameter("xsh", [P, 3, 96], F32, isOutput=False)
    cst = nc.declare_dram_parameter("cst", [P, 3, 2, NS], F32, isOutput=False)
    wcs = nc.declare_dram_parameter("wcs", [P, 2, NS], F32, isOutput=False)
    outp = nc.declare_dram_parameter("out", [P, 1], F32, isOutput=True)

    aL2 = float(alpha * L * L)
    s2aL = float(2.0 * alpha * L)
    Aj, Fs, Fd = _jastrow_consts()
    WMIN = float(1.0 - (1.0 - 1e-5) ** 2)   # lower clamp of w = 1 - x^2
    # -0.5 * sum_{s,i} ln(w_i) spread over the 2048 jastrow accum elements
    WBIAS = -0.5 * 2.0 * float(np.sum(np.log1p(np.arange(NS) * 2.0 ** -21))) \
        / (ND * N)

    with ExitStack() as ctx:
        tc = ctx.enter_context(tile.TileContext(nc))
        pool = ctx.enter_context(tc.tile_pool(name="main", bufs=1))

        # ---- loads & small constants ----
        ce = pool.tile([P, 3, 2, NS], F32, tag="ce")
        nc.default_dma_engine.dma_start(ce, cst[:])
        xe = pool.tile([P, 3, 96], F32, tag="xe")
        nc.default_dma_engine.dma_start(xe, xsh[:])

        biasc = pool.tile([P, 6], F32, tag="biasc")
        nc.gpsimd.memset(biasc[:, 0:1], -aL2)        # Exp image bias
        nc.gpsimd.memset(biasc[:, 1:2], -L / 2)      # Abs bias
        nc.gpsimd.memset(biasc[:, 2:3], L / 2)       # Square bias
        nc.gpsimd.memset(biasc[:, 3:4], 1e-37)       # Ln guard bias
        nc.gpsimd.memset(biasc[:, 4:5], WMIN)        # Ln bias for w
        nc.gpsimd.memset(biasc[:, 5:6], 1.0 - WMIN)  # Relu bias for w

        # static jastrow weights: wcol[d-1] = 1.0 except 0.5 for d=32
        wcol = pool.tile([P, ND, 1], F32, tag="wcol")
        nc.gpsimd.memset(wcol[:, 0:ND - 1], 1.0)
        nc.gpsimd.memset(wcol[:, ND - 1:ND], 0.5)

        # static per-pair exp scale Finv[d-1, i] = -1/F(spin(i), spin((i+d)%64))
        Finv = pool.tile([P, ND, N], F32, tag="Finv")
        nc.gpsimd.memset(Finv, -1.0 / Fd)
        for d in range(1, NS):   # d=32 row: all diff-spin, stays -1/Fd
            nc.gpsimd.memset(Finv[:, d - 1, 0:NS - d], -1.0 / Fs)
            nc.gpsimd.memset(Finv[:, d - 1, NS:N - d], -1.0 / Fs)

        # =========================================================
        # Slater matrices, column-major: A[p, s, j, i] = Phi[i, j]
        #   f_axis = e0 * (1 + p+ + p-),   Phi = fx*fy*fz
        # =========================================================
        B1 = pool.tile([P, 3, 2, NS, NS], F32, tag="B1")   # d -> p- -> wrap chain
        B2 = pool.tile([P, 3, 2, NS, NS], F32, tag="B2")   # d^2 -> e0 -> f
        B3 = pool.tile([P, 3, 2, NS, NS], F32, tag="B3")   # p+ -> q
        B4 = pool.tile([P, 3, ND, N], F32, tag="B4")       # jdx, later accum dump
        Abuf = pool.tile([P, 2, NS, NS], F32, tag="Abuf")  # f32 matrix (late steps)
        Abft = pool.tile([P, 2, NS, NS], BF16, tag="Abft")  # bf16 matrix (early)
        tprod = pool.tile([P, 2, NS, NS], F32, tag="tprod")

        ppart = list(xe.ap[0])

        # d[c,s,j,i] = x[c, s*32+i] - cen[c,s,j]  (per axis: 3 free dims)
        for c in range(3):
            xi = AP(xe.tensor, xe.offset + 96 * c,
                    [ppart, [NS, 2], [0, NS], [1, NS]])
            cj = AP(ce.tensor, ce.offset + 2 * NS * c,
                    [list(ce.ap[0]), [NS, 2], [1, NS], [0, NS]])
            nc.vector.tensor_tensor(B1[:, c], xi, cj, OP.subtract)

        B4f = B4.rearrange("p c a b -> p (c a b)")

        # squares first (one ACT table swap total), then per-axis exp chains
        # interleaved with DVE combines so neither engine stalls long
        dv, sqv, ppv = [], [], []
        for c in range(3):
            dv.append(B1[:, c].rearrange("p s a b -> p (s a b)"))
            sqv.append(B2[:, c].rearrange("p s a b -> p (s a b)"))
            ppv.append(B3[:, c].rearrange("p s a b -> p (s a b)"))
            nc.scalar.activation(sqv[c], dv[c], AF.Square)                 # d^2
        for c in range(3):
            nc.scalar.activation(ppv[c], dv[c], AF.Exp,
                                 bias=biasc[:, 0:1], scale=-s2aL)          # p+
            # jastrow banded differences jdx[c,dd,i] = x[c,i] - x[c,i+dd+1]
            # (per-axis DVE filler while ScalarE runs the slater chains)
            xib = AP(xe.tensor, xe.offset + 96 * c,
                     [ppart, [0, ND], [1, N]])
            xsk = AP(xe.tensor, xe.offset + 96 * c + 1,
                     [ppart, [1, ND], [1, N]])
            nc.vector.tensor_tensor(B4[:, c], xib, xsk, OP.subtract)
            nc.scalar.activation(dv[c], dv[c], AF.Exp,
                                 bias=biasc[:, 0:1], scale=s2aL)           # p- (in place)
            nc.scalar.activation(sqv[c], sqv[c], AF.Exp, scale=-alpha)     # e0 (in place)
            nc.vector.tensor_tensor(ppv[c], ppv[c], dv[c], OP.add)         # q = p+ + p-
            # f = (q + 1) * e0
            nc.vector.scalar_tensor_tensor(sqv[c], ppv[c], 1.0, sqv[c],
                                           OP.add, OP.mult)
        nc.vector.tensor_tensor(tprod, B2[:, 0], B2[:, 1], OP.mult)
        nc.vector.tensor_tensor(Abft, tprod, B2[:, 2], OP.mult)

        # jastrow wrap chain on ScalarE (queued after slater ACT work):
        # u = |dx|; b = |u - L/2|; wr2 = (L/2 - b)^2   (into B1, in place)
        B1f = B1.rearrange("p c s a b -> p (c s a b)")
        nc.scalar.activation(B1f, B4f, AF.Abs)
        nc.scalar.activation(B1f, B1f, AF.Abs, bias=biasc[:, 1:2])
        nc.scalar.activation(B1f, B1f, AF.Square,
                             bias=biasc[:, 2:3], scale=-1.0)              # wrapped^2

        # =========================================================
        # Batched Gaussian elimination w/ partial pivoting (both spins)
        # column-major A; pivot search on squared candidates
        # =========================================================
        c2b = pool.tile([P, 2, NS], F32, tag="c2b")
        c2m = pool.tile([P, 2, NS], F32, tag="c2m")
        indb = pool.tile([P, 2, NS], F32, tag="indb")
        indbb = pool.tile([P, 2, NS], BF16, tag="indbb")
        Mbarch = pool.tile([P, 2, NS], F32, tag="Mbarch")
        prs = pool.tile([P, 2, NS - 1], F32, tag="prs")
        prsb = pool.tile([P, 2, NS - 1, 2], BF16, tag="prsb")   # pair-duplicated
        rpv = pool.tile([P, 2, 1], F32, tag="rpv")
        maskw = pool.tile([P, 2, NS], F32, tag="maskw")
        prowall = pool.tile([P, 2, NS, NS], F32, tag="prowall")
        scr = pool.tile([P, 2, NS, NS], F32, tag="scr")
        scrb = pool.tile([P, 2, NS, NS], BF16, tag="scrb")

        # jastrow intermediates
        r2 = pool.tile([P, ND, N], F32, tag="r2")
        jq = pool.tile([P, ND, N], F32, tag="jq")      # 1/r -> G -> P
        jr = pool.tile([P, ND, N], F32, tag="jr")      # relu -> lnw -> 1/w
        jdec = pool.tile([P, ND, N], F32, tag="jdec")  # decay
        jes = pool.tile([P, ND, N], F32, tag="jes")    # ln r2 -> rF -> e_sel
        jsum = pool.tile([P, 1], F32, tag="jsum")
        labs = pool.tile([P, 2, NS], F32, tag="labs")
        ld1 = pool.tile([P, 1], F32, tag="ld1")

        # maskw: tie-break weights; used rows go negative (-2 trick) and are
        # never picked again (candidates are squares, so >= 0)
        nc.default_dma_engine.dma_start(maskw, wcs[:])

        # views of B1 (wrapped^2) per axis, shaped like r2
        wr2 = [AP(B1.tensor, B1.offset + 2048 * c,
                  [list(B1.ap[0]), [N, ND], [1, N]])
               for c in range(3)]

        def search_ops(k):
            """Squared-candidate argmax for step k (indicator into indb)."""
            A = Abft if k < KBF else Abuf
            colk = A[:, :, k, :]
            nc.vector.tensor_tensor(c2b, colk, colk, OP.mult)
            nc.vector.tensor_tensor(c2m, c2b, maskw, OP.mult)
            nc.vector.reduce_max(Mbarch[:, :, k], c2m, axis=AX.X)
            if k < NS - 1:
                ind = indbb if k < KBF else indb
                nc.vector.tensor_tensor(
                    ind, c2m,
                    Mbarch[:, :, k:k + 1].broadcast_to([P, 2, NS]), OP.is_equal
                )
                nc.vector.scalar_tensor_tensor(
                    maskw, ind, -2.0, maskw, OP.mult, OP.add
                )

        def extract_ops(k):
            """Pivot row extraction for step k into prowall[:, :, k, :T]."""
            T = NS - k
            if k < KBF:
                nc.vector.tensor_tensor(
                    scrb[:, :, :T, :],
                    Abft[:, :, k:, :],
                    indbb[:, :, None, :].broadcast_to([P, 2, T, NS]),
                    OP.mult,
                )
                nc.vector.reduce_sum(prowall[:, :, k, :T], scrb[:, :, :T, :],
                                     axis=AX.X)
            else:
                nc.vector.tensor_tensor(
                    scr[:, :, :T, :],
                    Abuf[:, :, k:, :],
                    indb[:, :, None, :].broadcast_to([P, 2, T, NS]),
                    OP.mult,
                )
                nc.vector.reduce_sum(prowall[:, :, k, :T], scr[:, :, :T, :],
                                     axis=AX.X)

        search_ops(0)
        extract_ops(0)
        for k in range(NS - 1):
            T = NS - k
            # jastrow fillers at fixed steps (DVE r2 adds, ScalarE chains,
            # GpSimd tail) -- all hidden inside the GE window
            if k == 2:
                nc.vector.tensor_tensor(r2, wr2[0], wr2[1], OP.add)
            elif k == 4:
                nc.vector.tensor_tensor(r2, r2, wr2[2], OP.add)
            elif k == 5:
                # grouped by activation family to minimise ACT table swaps
                nc.scalar.activation(jr, r2, AF.Relu, bias=biasc[:, 5:6],
                                     scale=-0.01)
                nc.scalar.activation(jes, r2, AF.Ln)                        # ln r2
                nc.scalar.activation(jr, jr, AF.Ln, bias=biasc[:, 4:5])     # ln w
                nc.scalar.activation(jq, jes, AF.Exp, scale=-0.5)           # 1/r
                nc.scalar.activation(r2, jes, AF.Exp, scale=0.5)            # r
                nc.scalar.activation(jr, jr, AF.Exp, scale=-1.0)            # 1/w
                nc.scalar.activation(jdec, jr, AF.Exp, bias=1.0, scale=-1.0)  # decay
            elif k == 14:
                nc.vector.tensor_tensor(jes, r2, Finv, OP.mult)         # -r/F_sel
            elif k == 15:
                nc.scalar.activation(jes, jes, AF.Exp)                  # e_sel
            elif k == 16:
                # preload the Ln activation table so the final logdet pass
                # does not pay a table swap on the critical tail
                nc.scalar.activation(ld1, biasc[:, 3:4], AF.Ln)
            elif k == 20:
                # w*(e_sel - 1)
                nc.vector.scalar_tensor_tensor(
                    jes, jes, -1.0, wcol.broadcast_to([P, ND, N]),
                    OP.add, OP.mult)
            elif k == 21:
                nc.vector.tensor_tensor(jq, jq, jdec, OP.mult)          # G = q*decay
            elif k == 23:
                nc.vector.tensor_tensor(jq, jq, jes, OP.mult)           # w*G*(e_sel-1)
            elif k == 27:
                # jast = Aj * sum(w*G*(e_sel-1))
                nc.scalar.activation(B4[:, 0], jq, AF.Copy, scale=Aj,
                                     bias=float(WBIAS), accum_out=jsum)

            # scaled pivot row: prs[j] = prow[j+1] * (1/pivot)
            nc.vector.reciprocal(rpv, prowall[:, :, k, 0:1])
            if k < KBF:
                # prs written pair-duplicated so the outer product keeps a
                # packed [stride1, 2] innermost on every operand (2x bf16)
                nc.vector.tensor_tensor(
                    prsb[:, :, :T - 1, :],
                    prowall[:, :, k, 1:T, None].broadcast_to([P, 2, T - 1, 2]),
                    rpv[:, :, :, None].broadcast_to([P, 2, T - 1, 2]),
                    OP.mult,
                )
                for sp in range(2):
                    colk5 = AP(Abft.tensor,
                               Abft.offset + sp * NS * NS + k * NS,
                               [list(Abft.ap[0]), [0, T - 1],
                                [2, NS // 2], [1, 2]])
                    prs5 = AP(prsb.tensor, prsb.offset + sp * 2 * (NS - 1),
                              [list(prsb.ap[0]), [2, T - 1],
                               [0, NS // 2], [1, 2]])
                    scr5 = AP(scrb.tensor, scrb.offset + sp * NS * NS,
                              [list(scrb.ap[0]), [NS, T - 1],
                               [2, NS // 2], [1, 2]])
                    nc.vector.tensor_tensor(scr5, colk5, prs5, OP.mult)
                A, scrc = Abft, scrb
            else:
                A, scrc = Abuf, scr
                nc.vector.tensor_tensor(
                    prs[:, :, :T - 1],
                    prowall[:, :, k, 1:T],
                    rpv.broadcast_to([P, 2, T - 1]),
                    OP.mult,
                )
                colk = A[:, :, k, :]
                # outer product scr[j,i] = colk[i] * prs[j]
                nc.vector.tensor_tensor(
                    scrc[:, :, :T - 1, :],
                    colk[:, :, None, :].broadcast_to([P, 2, T - 1, NS]),
                    prs[:, :, :T - 1, None].broadcast_to([P, 2, T - 1, NS]),
                    OP.mult,
                )
            # one flat subtract over cols k+1.. ; at the precision transition
            # the result lands directly in the f32 matrix (no separate copy)
            Aout = Abuf if k + 1 == KBF else A
            nc.vector.tensor_tensor(
                Aout[:, :, k + 1:, :].rearrange("p s a b -> p s (a b)"),
                A[:, :, k + 1:, :].rearrange("p s a b -> p s (a b)"),
                scrc[:, :, :T - 1, :].rearrange("p s a b -> p s (a b)"),
                OP.subtract,
            )
            search_ops(k + 1)
            if k + 1 < NS - 1:
                extract_ops(k + 1)

        # =========================================================
        # logdet:  Mb_k = piv_k^2 * w_ipiv, and every row is pivoted exactly
        # once, so  ld = 0.5*(sum_k ln Mb_k - sum_i ln w_i)  with the weight
        # constant folded into jsum's accumulation bias.
        # =========================================================
        nc.scalar.activation(labs, Mbarch, AF.Ln, bias=biasc[:, 3:4])
        nc.vector.reduce_sum(ld1, labs.rearrange("p s a -> p (s a)"), axis=AX.X)
        ob = pool.tile([P, 1], F32, tag="ob")
        nc.vector.scalar_tensor_tensor(ob, ld1, 0.5, jsum, OP.mult, OP.add)
        nc.default_dma_engine.dma_start(outp[:], ob)

    nc.finalize()
    return nc


_CACHE = {}


def _get_built(alpha: float):
    key = round(alpha, 9)
    if key not in _CACHE:
        _CACHE[key] = _build(alpha)
    return _CACHE[key]


def _make_inputs(walkerRs: np.ndarray):
    cu, cd = _centers()
    cen = np.stack([cu, cd], 0)                   # (2, NS, 3)
    cst = np.ascontiguousarray(
        np.broadcast_to(cen.transpose(2, 0, 1)[None], (P, 3, 2, NS))
    ).astype(np.float32)
    w = (1.0 + np.arange(NS) * 2.0 ** -21).astype(np.float32)
    wcs = np.ascontiguousarray(
        np.broadcast_to(w[None, None, :], (P, 2, NS))
    ).astype(np.float32)
    in_maps = []
    for c in range(NCORES):
        sh = walkerRs[c * P:(c + 1) * P] % L      # (P, N, 3)
        xt = sh.transpose(0, 2, 1)                # (P, 3, N)
        x2 = np.concatenate([xt, xt[:, :, :NS]], axis=2)   # (P, 3, 96)
        in_maps.append({"xsh": np.ascontiguousarray(x2).astype(np.float32),
                        "cst": cst, "wcs": wcs})
    return in_maps


def kernel(walkerRs: np.ndarray, log_alpha: np.ndarray, _trace=False):
    walkerRs = np.asarray(walkerRs, dtype=np.float32)
    la = float(np.asarray(log_alpha))
    alpha = float(np.clip(np.exp(la), 55.0 / L ** 2, 300.0 / L ** 2))
    nc = _get_built(alpha)
    in_maps = _make_inputs(walkerRs)
    res = None
    for attempt in range(3):
        try:
            res = run_bass_kernel_spmd(nc, in_maps, list(range(NCORES)),
                                       trace=_trace)
            break
        except Exception:
            # transient NRT "device unrecoverable" after a prior bad run
            if attempt == 2:
                raise
            import time as _time
            _time.sleep(15)
    out = np.concatenate([res.results[i]["out"][:, 0] for i in range(NCORES)])
    if _trace:
        return out.astype(np.float32), res
    return out.astype(np.float32)


# revision 61
# speedup vs baseline: 1.0047x; 1.0047x over previous
"""Trainium2 Bass kernel for LogWignerCrystalSlaterFixedCYJastrow.

Computes, per walker (batch of 1024, 64 electrons in 3D, box L=20):
    out = logdet(Phi_up) + logdet(Phi_dn) + jastrow
where Phi_s are 32x32 Gaussian-orbital Slater matrices over 27 periodic
images (collapsed analytically to a separable per-axis 3-image sum), and
jastrow is a Coulomb-Yukawa pair sum with minimum-image wrapping.

Strategy: pure data parallel over 8 NeuronCores, 128 walkers per core,
one walker per SBUF partition.  The two 32x32 slogdets per walker are done
with a batched, in-SBUF Gaussian elimination with partial pivoting
(pivot row selected/extracted with indicator arithmetic -- no data
dependent control flow, identical instruction stream for all walkers).

v2 structure vs the earlier kernel:
 - Jastrow over a banded pair set (i, i+d mod 64), d=1..32 (2112 pairs/
   walker vs 4096 matrix entries): each unordered pair exactly once (the
   d=32 column weighted 0.5).  No diagonal masking needed.  Same/diff-spin
   exp selection folded into a single Exp via a static per-pair -1/F tile
   (GpSimd memsets at the idle head): e_sel = exp(-r/F_sel).
 - All jastrow transcendentals run on ScalarE inside the GE window; its
   elementwise tail is spread across GE steps as DVE fillers.  (The tail
   stays fp32: ScalarE activations writing bf16 outputs produced wrong
   values on this hardware, so bf16 is restricted to the GE matrix ops.)
 - Mixed precision GE: the first KBF=11 elimination steps run on a bf16
   matrix (pivot search candidates and pivot rows stay fp32), which gets
   2x DVE throughput on the packed extraction-mult and update-subtract;
   the transition step's subtract writes the trailing block directly in
   fp32.  The bf16 rank-1 outer product also reaches 2x via pair
   duplication: prs is written as adjacent pairs so every operand of the
   per-spin outer keeps a packed [stride-1 x 2] innermost dim (a plain
   broadcast has innermost stride 0, which disables the fast mode).  Measured max rel err 1.59e-2 on the fixed harness inputs (gate
   2e-2, fully deterministic); the fp32-only variant measures 1.25e-4 but
   is ~12us slower.
 - logdet read off the pivot-search maxima: Mb_k = piv_k^2 * w_ipiv and
   every row pivots exactly once, so ld = 0.5*(sum ln Mb_k - sum ln w_i),
   with the weight constant folded into the jastrow accumulation bias and
   the Ln table preloaded during the GE so the tail pays no swap.
"""

import os
import sys
import numpy as np
from contextlib import ExitStack

for _p in ("/opt/trn_rl_repo", "/opt/pypackages"):
    if _p not in sys.path:
        sys.path.append(_p)

import concourse.bass as bass
import concourse.bacc as bacc
import concourse.mybir as mybir
import concourse.tile as tile
from concourse.bass import AP
from concourse.bass_utils import run_bass_kernel_spmd

P = 128          # partitions = walkers per core
NCORES = 8
B = 1024
N = 64           # electrons per walker
NS = 32          # electrons / orbitals per spin
ND = 32          # banded jastrow offsets (d = 1..32)
L = 20.0
F32 = mybir.dt.float32
BF16 = mybir.dt.bfloat16
AF = mybir.ActivationFunctionType
OP = mybir.AluOpType
AX = mybir.AxisListType
KBF = 12         # GE steps 0..KBF-1 run on a bf16 matrix (2x DVE on packed ops)


def _centers():
    n = 1
    while n ** 3 < NS:
        n += 1
    a = L / n
    coords = np.linspace(0.0, L - a, n)
    grid = np.stack(np.meshgrid(coords, coords, coords, indexing="ij"), axis=-1)
    grid = grid.reshape(-1, 3)
    cu = grid[:NS].astype(np.float32)
    cd = (grid + a / 2)[:NS].astype(np.float32)
    return cu, cd


def _jastrow_consts():
    dens = np.float32(N / L ** 3)
    A = np.float32(1.0) / np.sqrt(np.float32(4 * np.pi) * dens, dtype=np.float32)
    Fs = np.sqrt(np.float32(2.0) * A, dtype=np.float32)
    Fd = np.sqrt(A, dtype=np.float32)
    return float(A), float(Fs), float(Fd)


def _build(alpha: float) -> bass.Bass:
    nc = bacc.Bacc()
    # xsh: electron coords doubled along the electron axis (96 = 64 + 32)
    xsh = nc.declare_dram_parameter("xsh", [P, 3, 96], F32, isOutput=False)
    cst = nc.declare_dram_parameter("cst", [P, 3, 2, NS], F32, isOutput=False)
    wcs = nc.declare_dram_parameter("wcs", [P, 2, NS], F32, isOutput=False)
    outp = nc.declare_dram_parameter("out", [P, 1], F32, isOutput=True)

    aL2 = float(alpha * L * L)
    s2aL = float(2.0 * alpha * L)
    Aj, Fs, Fd = _jastrow_consts()
    WMIN = float(1.0 - (1.0 - 1e-5) ** 2)   # lower clamp of w = 1 - x^2
    # -0.5 * sum_{s,i} ln(w_i) spread over the 2048 jastrow accum elements
    WBIAS = -0.5 * 2.0 * float(np.sum(np.log1p(np.arange(NS) * 2.0 ** -21))) \
        / (ND * N)

    with ExitStack() as ctx:
        tc = ctx.enter_context(tile.TileContext(nc))
        pool = ctx.enter_context(tc.tile_pool(name="main", bufs=1))

        # ---- loads & small constants ----
        ce = pool.tile([P, 3, 2, NS], F32, tag="ce")
        nc.default_dma_engine.dma_start(ce, cst[:])
        xe = pool.tile([P, 3, 96], F32, tag="xe")
        nc.default_dma_engine.dma_start(xe, xsh[:])

        biasc = pool.tile([P, 6], F32, tag="biasc")
        nc.gpsimd.memset(biasc[:, 0:1], -aL2)        # Exp image bias
        nc.gpsimd.memset(biasc[:, 1:2], -L / 2)      # Abs bias
        nc.gpsimd.memset(biasc[:, 2:3], L / 2)       # Square bias
        nc.gpsimd.memset(biasc[:, 3:4], 1e-37)       # Ln guard bias
        nc.gpsimd.memset(biasc[:, 4:5], WMIN)        # Ln bias for w
        nc.gpsimd.memset(biasc[:, 5:6], 1.0 - WMIN)  # Relu bias for w

        # static jastrow weights: wcol[d-1] = 1.0 except 0.5 for d=32
        wcol = pool.tile([P, ND, 1], F32, tag="wcol")
        nc.gpsimd.memset(wcol[:, 0:ND - 1], 1.0)
        nc.gpsimd.memset(wcol[:, ND - 1:ND], 0.5)

        # static per-pair exp scale Finv[d-1, i] = -1/F(spin(i), spin((i+d)%64))
        Finv = pool.tile([P, ND, N], F32, tag="Finv")
        nc.gpsimd.memset(Finv, -1.0 / Fd)
        for d in range(1, NS):   # d=32 row: all diff-spin, stays -1/Fd
            nc.gpsimd.memset(Finv[:, d - 1, 0:NS - d], -1.0 / Fs)
            nc.gpsimd.memset(Finv[:, d - 1, NS:N - d], -1.0 / Fs)

        # =========================================================
        # Slater matrices, column-major: A[p, s, j, i] = Phi[i, j]
        #   f_axis = e0 * (1 + p+ + p-),   Phi = fx*fy*fz
        # =========================================================
        B1 = pool.tile([P, 3, 2, NS, NS], F32, tag="B1")   # d -> p- -> wrap chain
        B2 = pool.tile([P, 3, 2, NS, NS], F32, tag="B2")   # d^2 -> e0 -> f
        B3 = pool.tile([P, 3, 2, NS, NS], F32, tag="B3")   # p+ -> q
        B4 = pool.tile([P, 3, ND, N], F32, tag="B4")       # jdx, later accum dump
        Abuf = pool.tile([P, 2, NS, NS], F32, tag="Abuf")  # f32 matrix (late steps)
        Abft = pool.tile([P, 2, NS, NS], BF16, tag="Abft")  # bf16 matrix (early)
        tprod = pool.tile([P, 2, NS, NS], F32, tag="tprod")

        ppart = list(xe.ap[0])

        # d[c,s,j,i] = x[c, s*32+i] - cen[c,s,j]  (per axis: 3 free dims)
        for c in range(3):
            xi = AP(xe.tensor, xe.offset + 96 * c,
                    [ppart, [NS, 2], [0, NS], [1, NS]])
            cj = AP(ce.tensor, ce.offset + 2 * NS * c,
                    [list(ce.ap[0]), [NS, 2], [1, NS], [0, NS]])
            nc.vector.tensor_tensor(B1[:, c], xi, cj, OP.subtract)

        B4f = B4.rearrange("p c a b -> p (c a b)")

        # per-axis ScalarE chains interleaved with DVE combines so neither
        # engine stalls long at the head
        for c in range(3):
            d_c = B1[:, c].rearrange("p s a b -> p (s a b)")
            sq_c = B2[:, c].rearrange("p s a b -> p (s a b)")
            pp_c = B3[:, c].rearrange("p s a b -> p (s a b)")
            nc.scalar.activation(sq_c, d_c, AF.Square)                     # d^2
            nc.scalar.activation(pp_c, d_c, AF.Exp,
                                 bias=biasc[:, 0:1], scale=-s2aL)          # p+
            # jastrow banded differences jdx[c,dd,i] = x[c,i] - x[c,i+dd+1]
            # (per-axis DVE filler while ScalarE runs the slater chains)
            xib = AP(xe.tensor, xe.offset + 96 * c,
                     [ppart, [0, ND], [1, N]])
            xsk = AP(xe.tensor, xe.offset + 96 * c + 1,
                     [ppart, [1, ND], [1, N]])
            nc.vector.tensor_tensor(B4[:, c], xib, xsk, OP.subtract)
            nc.scalar.activation(d_c, d_c, AF.Exp,
                                 bias=biasc[:, 0:1], scale=s2aL)           # p- (in place)
            nc.scalar.activation(sq_c, sq_c, AF.Exp, scale=-alpha)         # e0 (in place)
            nc.vector.tensor_tensor(pp_c, pp_c, d_c, OP.add)               # q = p+ + p-
            # f = (q + 1) * e0
            nc.vector.scalar_tensor_tensor(sq_c, pp_c, 1.0, sq_c,
                                           OP.add, OP.mult)
        nc.vector.tensor_tensor(tprod, B2[:, 0], B2[:, 1], OP.mult)
        nc.vector.tensor_tensor(Abft, tprod, B2[:, 2], OP.mult)

        # jastrow wrap chain on ScalarE (queued after slater ACT work):
        # u = |dx|; b = |u - L/2|; wr2 = (L/2 - b)^2   (into B1, in place)
        B1f = B1.rearrange("p c s a b -> p (c s a b)")
        nc.scalar.activation(B1f, B4f, AF.Abs)
        nc.scalar.activation(B1f, B1f, AF.Abs, bias=biasc[:, 1:2])
        nc.scalar.activation(B1f, B1f, AF.Square,
                             bias=biasc[:, 2:3], scale=-1.0)              # wrapped^2

        # =========================================================
        # Batched Gaussian elimination w/ partial pivoting (both spins)
        # column-major A; pivot search on squared candidates
        # =========================================================
        c2b = pool.tile([P, 2, NS], F32, tag="c2b")
        c2m = pool.tile([P, 2, NS], F32, tag="c2m")
        indb = pool.tile([P, 2, NS], F32, tag="indb")
        indbb = pool.tile([P, 2, NS], BF16, tag="indbb")
        Mbarch = pool.tile([P, 2, NS], F32, tag="Mbarch")
        prs = pool.tile([P, 2, NS - 1], F32, tag="prs")
        prsb = pool.tile([P, 2, NS - 1, 2], BF16, tag="prsb")   # pair-duplicated
        rpv = pool.tile([P, 2, 1], F32, tag="rpv")
        maskw = pool.tile([P, 2, NS], F32, tag="maskw")
        prowall = pool.tile([P, 2, NS, NS], F32, tag="prowall")
        scr = pool.tile([P, 2, NS, NS], F32, tag="scr")
        scrb = pool.tile([P, 2, NS, NS], BF16, tag="scrb")

        # jastrow intermediates
        r2 = pool.tile([P, ND, N], F32, tag="r2")
        jq = pool.tile([P, ND, N], F32, tag="jq")      # 1/r -> G -> P
        jr = pool.tile([P, ND, N], F32, tag="jr")      # relu -> lnw -> 1/w
        jdec = pool.tile([P, ND, N], F32, tag="jdec")  # decay
        jes = pool.tile([P, ND, N], F32, tag="jes")    # ln r2 -> rF -> e_sel
        jsum = pool.tile([P, 1], F32, tag="jsum")
        labs = pool.tile([P, 2, NS], F32, tag="labs")
        ld1 = pool.tile([P, 1], F32, tag="ld1")

        # maskw: tie-break weights; used rows go negative (-2 trick) and are
        # never picked again (candidates are squares, so >= 0)
        nc.default_dma_engine.dma_start(maskw, wcs[:])

        # views of B1 (wrapped^2) per axis, shaped like r2
        wr2 = [AP(B1.tensor, B1.offset + 2048 * c,
                  [list(B1.ap[0]), [N, ND], [1, N]])
               for c in range(3)]

        def search_ops(k):
            """Squared-candidate argmax for step k (indicator into indb)."""
            A = Abft if k < KBF else Abuf
            colk = A[:, :, k, :]
            nc.vector.tensor_tensor(c2b, colk, colk, OP.mult)
            nc.vector.tensor_tensor(c2m, c2b, maskw, OP.mult)
            nc.vector.reduce_max(Mbarch[:, :, k], c2m, axis=AX.X)
            if k < NS - 1:
                ind = indbb if k < KBF else indb
                nc.vector.tensor_tensor(
                    ind, c2m,
                    Mbarch[:, :, k:k + 1].broadcast_to([P, 2, NS]), OP.is_equal
                )
                nc.vector.scalar_tensor_tensor(
                    maskw, ind, -2.0, maskw, OP.mult, OP.add
                )

        def extract_ops(k):
            """Pivot row extraction for step k into prowall[:, :, k, :T]."""
            T = NS - k
            if k < KBF:
                nc.vector.tensor_tensor(
                    scrb[:, :, :T, :],
                    Abft[:, :, k:, :],
                    indbb[:, :, None, :].broadcast_to([P, 2, T, NS]),
                    OP.mult,
                )
                nc.vector.reduce_sum(prowall[:, :, k, :T], scrb[:, :, :T, :],
                                     axis=AX.X)
            else:
                nc.vector.tensor_tensor(
                    scr[:, :, :T, :],
                    Abuf[:, :, k:, :],
                    indb[:, :, None, :].broadcast_to([P, 2, T, NS]),
                    OP.mult,
                )
                nc.vector.reduce_sum(prowall[:, :, k, :T], scr[:, :, :T, :],
                                     axis=AX.X)

        search_ops(0)
        extract_ops(0)
        for k in range(NS - 1):
            T = NS - k
            # jastrow fillers at fixed steps (DVE r2 adds, ScalarE chains,
            # GpSimd tail) -- all hidden inside the GE window
            if k == 2:
                nc.vector.tensor_tensor(r2, wr2[0], wr2[1], OP.add)
            elif k == 4:
                nc.vector.tensor_tensor(r2, r2, wr2[2], OP.add)
            elif k == 5:
                # grouped by activation family to minimise ACT table swaps
                nc.scalar.activation(jr, r2, AF.Relu, bias=biasc[:, 5:6],
                                     scale=-0.01)
                nc.scalar.activation(jes, r2, AF.Ln)                        # ln r2
                nc.scalar.activation(jr, jr, AF.Ln, bias=biasc[:, 4:5])     # ln w
                nc.scalar.activation(jq, jes, AF.Exp, scale=-0.5)           # 1/r
                nc.scalar.activation(r2, jes, AF.Exp, scale=0.5)            # r
                nc.scalar.activation(jr, jr, AF.Exp, scale=-1.0)            # 1/w
                nc.scalar.activation(jdec, jr, AF.Exp, bias=1.0, scale=-1.0)  # decay
            elif k == 14:
                nc.vector.tensor_tensor(jes, r2, Finv, OP.mult)         # -r/F_sel
            elif k == 15:
                nc.scalar.activation(jes, jes, AF.Exp)                  # e_sel
            elif k == 16:
                # preload the Ln activation table so the final logdet pass
                # does not pay a table swap on the critical tail
                nc.scalar.activation(ld1, biasc[:, 3:4], AF.Ln)
            elif k == 20:
                # w*(e_sel - 1)
                nc.vector.scalar_tensor_tensor(
                    jes, jes, -1.0, wcol.broadcast_to([P, ND, N]),
                    OP.add, OP.mult)
            elif k == 21:
                nc.vector.tensor_tensor(jq, jq, jdec, OP.mult)          # G = q*decay
            elif k == 23:
                nc.vector.tensor_tensor(jq, jq, jes, OP.mult)           # w*G*(e_sel-1)
            elif k == 27:
                # jast = Aj * sum(w*G*(e_sel-1))
                nc.scalar.activation(B4[:, 0], jq, AF.Copy, scale=Aj,
                                     bias=float(WBIAS), accum_out=jsum)

            # scaled pivot row: prs[j] = prow[j+1] * (1/pivot)
            nc.vector.reciprocal(rpv, prowall[:, :, k, 0:1])
            if k < KBF:
                # prs written pair-duplicated so the outer product keeps a
                # packed [stride1, 2] innermost on every operand (2x bf16)
                nc.vector.tensor_tensor(
                    prsb[:, :, :T - 1, :],
                    prowall[:, :, k, 1:T, None].broadcast_to([P, 2, T - 1, 2]),
                    rpv[:, :, :, None].broadcast_to([P, 2, T - 1, 2]),
                    OP.mult,
                )
                for sp in range(2):
                    colk5 = AP(Abft.tensor,
                               Abft.offset + sp * NS * NS + k * NS,
                               [list(Abft.ap[0]), [0, T - 1],
                                [2, NS // 2], [1, 2]])
                    prs5 = AP(prsb.tensor, prsb.offset + sp * 2 * (NS - 1),
                              [list(prsb.ap[0]), [2, T - 1],
                               [0, NS // 2], [1, 2]])
                    scr5 = AP(scrb.tensor, scrb.offset + sp * NS * NS,
                              [list(scrb.ap[0]), [NS, T - 1],
                               [2, NS // 2], [1, 2]])
                    nc.vector.tensor_tensor(scr5, colk5, prs5, OP.mult)
                A, scrc = Abft, scrb
            else:
                A, scrc = Abuf, scr
                nc.vector.tensor_tensor(
                    prs[:, :, :T - 1],
                    prowall[:, :, k, 1:T],
                    rpv.broadcast_to([P, 2, T - 1]),
                    OP.mult,
                )
                colk = A[:, :, k, :]
                # outer product scr[j,i] = colk[i] * prs[j]
                nc.vector.tensor_tensor(
                    scrc[:, :, :T - 1, :],
                    colk[:, :, None, :].broadcast_to([P, 2, T - 1, NS]),
                    prs[:, :, :T - 1, None].broadcast_to([P, 2, T - 1, NS]),
                    OP.mult,
                )
            # one flat subtract over cols k+1.. ; at the precision transition
            # the result lands directly in the f32 matrix (no separate copy)
            Aout = Abuf if k + 1 == KBF else A
            nc.vector.tensor_tensor(
                Aout[:, :, k + 1:, :].rearrange("p s a b -> p s (a b)"),
                A[:, :, k + 1:, :].rearrange("p s a b -> p s (a b)"),
                scrc[:, :, :T - 1, :].rearrange("p s a b -> p s (a b)"),
                OP.subtract,
            )
            search_ops(k + 1)
            if k + 1 < NS - 1:
                extract_ops(k + 1)

        # =========================================================
        # logdet:  Mb_k = piv_k^2 * w_ipiv, and every row is pivoted exactly
        # once, so  ld = 0.5*(sum_k ln Mb_k - sum_i ln w_i)  with the weight
        # constant folded into jsum's accumulation bias.
        # =========================================================
        nc.scalar.activation(labs, Mbarch, AF.Ln, bias=biasc[:, 3:4])
        nc.vector.reduce_sum(ld1, labs.rearrange("p s a -> p (s a)"), axis=AX.X)
        ob = pool.tile([P, 1], F32, tag="ob")
        nc.vector.scalar_tensor_tensor(ob, ld1, 0.5, jsum, OP.mult, OP.add)
        nc.default_dma_engine.dma_start(outp[:], ob)

    nc.finalize()
    return nc


_CACHE = {}


def _get_built(alpha: float):
    key = round(alpha, 9)
    if key not in _CACHE:
        _CACHE[key] = _build(alpha)
    return _CACHE[key]


def _make_inputs(walkerRs: np.ndarray):
    cu, cd = _centers()
    cen = np.stack([cu, cd], 0)                   # (2, NS, 3)
    cst = np.ascontiguousarray(
        np.broadcast_to(cen.transpose(2, 0, 1)[None], (P, 3, 2, NS))
    ).astype(np.float32)
    w = (1.0 + np.arange(NS) * 2.0 ** -21).astype(np.float32)
    wcs = np.ascontiguousarray(
        np.broadcast_to(w[None, None, :], (P, 2, NS))
    ).astype(np.float32)
    in_maps = []
    for c in range(NCORES):
        sh = walkerRs[c * P:(c + 1) * P] % L      # (P, N, 3)
        xt = sh.transpose(0, 2, 1)                # (P, 3, N)
        x2 = np.concatenate([xt, xt[:, :, :NS]], axis=2)   # (P, 3, 96)
        in_maps.append({"xsh": np.ascontiguousarray(x2).astype(np.float32),
                        "cst": cst, "wcs": wcs})
    return in_maps


def kernel(walkerRs: np.ndarray, log_alpha: np.ndarray, _trace=False):
    walkerRs = np.asarray(walkerRs, dtype=np.float32)
    la = float(np.asarray(log_alpha))
    alpha = float(np.clip(np.exp(la), 55.0 / L ** 2, 300.0 / L ** 2))
    nc = _get_built(alpha)
    in_maps = _make_inputs(walkerRs)
    res = None
    for attempt in range(3):
        try:
            res = run_bass_kernel_spmd(nc, in_maps, list(range(NCORES)),
                                       trace=_trace)
            break
        except Exception:
            # transient NRT "device unrecoverable" after a prior bad run
            if attempt == 2:
                raise
            import time as _time
            _time.sleep(15)
    out = np.concatenate([res.results[i]["out"][:, 0] for i in range(NCORES)])
    if _trace:
        return out.astype(np.float32), res
    return out.astype(np.float32)


# revision 62
# speedup vs baseline: 1.0090x; 1.0043x over previous
"""Trainium2 Bass kernel for LogWignerCrystalSlaterFixedCYJastrow.

Computes, per walker (batch of 1024, 64 electrons in 3D, box L=20):
    out = logdet(Phi_up) + logdet(Phi_dn) + jastrow
where Phi_s are 32x32 Gaussian-orbital Slater matrices over 27 periodic
images (collapsed analytically to a separable per-axis 3-image sum), and
jastrow is a Coulomb-Yukawa pair sum with minimum-image wrapping.

Strategy: pure data parallel over 8 NeuronCores, 128 walkers per core,
one walker per SBUF partition.  The two 32x32 slogdets per walker are done
with a batched, in-SBUF Gaussian elimination with partial pivoting
(pivot row selected/extracted with indicator arithmetic -- no data
dependent control flow, identical instruction stream for all walkers).

v2 structure vs the earlier kernel:
 - Jastrow over a banded pair set (i, i+d mod 64), d=1..32 (2112 pairs/
   walker vs 4096 matrix entries): each unordered pair exactly once (the
   d=32 column weighted 0.5).  No diagonal masking needed.  Same/diff-spin
   exp selection folded into a single Exp via a static per-pair -1/F tile
   (GpSimd memsets at the idle head): e_sel = exp(-r/F_sel).
 - All jastrow transcendentals run on ScalarE inside the GE window; its
   elementwise tail is spread across GE steps as DVE fillers.  (The tail
   stays fp32: ScalarE activations writing bf16 outputs produced wrong
   values on this hardware, so bf16 is restricted to the GE matrix ops.)
 - Mixed precision GE: the first KBF=11 elimination steps run on a bf16
   matrix (pivot search candidates and pivot rows stay fp32), which gets
   2x DVE throughput on the packed extraction-mult and update-subtract;
   the transition step's subtract writes the trailing block directly in
   fp32.  The bf16 rank-1 outer product also reaches 2x via pair
   duplication: prs is written as adjacent pairs so every operand of the
   per-spin outer keeps a packed [stride-1 x 2] innermost dim (a plain
   broadcast has innermost stride 0, which disables the fast mode).  Measured max rel err 1.59e-2 on the fixed harness inputs (gate
   2e-2, fully deterministic); the fp32-only variant measures 1.25e-4 but
   is ~12us slower.
 - logdet read off the pivot-search maxima: Mb_k = piv_k^2 * w_ipiv and
   every row pivots exactly once, so ld = 0.5*(sum ln Mb_k - sum ln w_i),
   with the weight constant folded into the jastrow accumulation bias and
   the Ln table preloaded during the GE so the tail pays no swap.
"""

import os
import sys
import numpy as np
from contextlib import ExitStack

for _p in ("/opt/trn_rl_repo", "/opt/pypackages"):
    if _p not in sys.path:
        sys.path.append(_p)

import concourse.bass as bass
import concourse.bacc as bacc
import concourse.mybir as mybir
import concourse.tile as tile
from concourse.bass import AP
from concourse.bass_utils import run_bass_kernel_spmd

P = 128          # partitions = walkers per core
NCORES = 8
B = 1024
N = 64           # electrons per walker
NS = 32          # electrons / orbitals per spin
ND = 32          # banded jastrow offsets (d = 1..32)
L = 20.0
F32 = mybir.dt.float32
BF16 = mybir.dt.bfloat16
AF = mybir.ActivationFunctionType
OP = mybir.AluOpType
AX = mybir.AxisListType
KBF = 12         # GE steps 0..KBF-1 run on a bf16 matrix (2x DVE on packed ops)


def _centers():
    n = 1
    while n ** 3 < NS:
        n += 1
    a = L / n
    coords = np.linspace(0.0, L - a, n)
    grid = np.stack(np.meshgrid(coords, coords, coords, indexing="ij"), axis=-1)
    grid = grid.reshape(-1, 3)
    cu = grid[:NS].astype(np.float32)
    cd = (grid + a / 2)[:NS].astype(np.float32)
    return cu, cd


def _jastrow_consts():
    dens = np.float32(N / L ** 3)
    A = np.float32(1.0) / np.sqrt(np.float32(4 * np.pi) * dens, dtype=np.float32)
    Fs = np.sqrt(np.float32(2.0) * A, dtype=np.float32)
    Fd = np.sqrt(A, dtype=np.float32)
    return float(A), float(Fs), float(Fd)


def _build(alpha: float) -> bass.Bass:
    nc = bacc.Bacc()
    # xsh: electron coords doubled along the electron axis (96 = 64 + 32)
    xsh = nc.declare_dram_parameter("xsh", [P, 3, 96], F32, isOutput=False)
    cst = nc.declare_dram_parameter("cst", [P, 3, 2, NS], F32, isOutput=False)
    wcs = nc.declare_dram_parameter("wcs", [P, 2, NS], F32, isOutput=False)
    outp = nc.declare_dram_parameter("out", [P, 1], F32, isOutput=True)

    aL2 = float(alpha * L * L)
    s2aL = float(2.0 * alpha * L)
    Aj, Fs, Fd = _jastrow_consts()
    WMIN = float(1.0 - (1.0 - 1e-5) ** 2)   # lower clamp of w = 1 - x^2
    # -0.5 * sum_{s,i} ln(w_i) spread over the 2048 jastrow accum elements
    WBIAS = -0.5 * 2.0 * float(np.sum(np.log1p(np.arange(NS) * 2.0 ** -21))) \
        / (ND * N)

    with ExitStack() as ctx:
        tc = ctx.enter_context(tile.TileContext(nc))
        pool = ctx.enter_context(tc.tile_pool(name="main", bufs=1))

        # ---- loads & small constants ----
        ce = pool.tile([P, 3, 2, NS], F32, tag="ce")
        nc.default_dma_engine.dma_start(ce, cst[:])
        xe = pool.tile([P, 3, 96], F32, tag="xe")
        nc.default_dma_engine.dma_start(xe, xsh[:])

        biasc = pool.tile([P, 6], F32, tag="biasc")
        nc.gpsimd.memset(biasc[:, 0:1], -aL2)        # Exp image bias
        nc.gpsimd.memset(biasc[:, 1:2], -L / 2)      # Abs bias
        nc.gpsimd.memset(biasc[:, 2:3], L / 2)       # Square bias
        nc.gpsimd.memset(biasc[:, 3:4], 1e-37)       # Ln guard bias
        nc.gpsimd.memset(biasc[:, 4:5], WMIN)        # Ln bias for w
        nc.gpsimd.memset(biasc[:, 5:6], 1.0 - WMIN)  # Relu bias for w

        # static jastrow weights: wcol[d-1] = 1.0 except 0.5 for d=32
        wcol = pool.tile([P, ND, 1], F32, tag="wcol")
        nc.gpsimd.memset(wcol[:, 0:ND - 1], 1.0)
        nc.gpsimd.memset(wcol[:, ND - 1:ND], 0.5)

        # static per-pair exp scale Finv[d-1, i] = -1/F(spin(i), spin((i+d)%64))
        Finv = pool.tile([P, ND, N], F32, tag="Finv")
        nc.gpsimd.memset(Finv, -1.0 / Fd)
        for d in range(1, NS):   # d=32 row: all diff-spin, stays -1/Fd
            nc.gpsimd.memset(Finv[:, d - 1, 0:NS - d], -1.0 / Fs)
            nc.gpsimd.memset(Finv[:, d - 1, NS:N - d], -1.0 / Fs)

        # =========================================================
        # Slater matrices, column-major: A[p, s, j, i] = Phi[i, j]
        #   f_axis = e0 * (1 + p+ + p-),   Phi = fx*fy*fz
        # =========================================================
        B1 = pool.tile([P, 3, 2, NS, NS], F32, tag="B1")   # d -> p- -> wrap chain
        B2 = pool.tile([P, 3, 2, NS, NS], F32, tag="B2")   # d^2 -> e0 -> f
        B3 = pool.tile([P, 3, 2, NS, NS], F32, tag="B3")   # p+ -> q
        B4 = pool.tile([P, 3, ND, N], F32, tag="B4")       # jdx, later accum dump
        Abuf = pool.tile([P, 2, NS, NS], F32, tag="Abuf")  # f32 matrix (late steps)
        Abft = pool.tile([P, 2, NS, NS], BF16, tag="Abft")  # bf16 matrix (early)
        fb = pool.tile([P, 3, 2, NS, NS], BF16, tag="fb")   # per-axis factors
        tprod = pool.tile([P, 2, NS, NS], BF16, tag="tprod")

        ppart = list(xe.ap[0])

        # d[c,s,j,i] = x[c, s*32+i] - cen[c,s,j]  (per axis: 3 free dims)
        for c in range(3):
            xi = AP(xe.tensor, xe.offset + 96 * c,
                    [ppart, [NS, 2], [0, NS], [1, NS]])
            cj = AP(ce.tensor, ce.offset + 2 * NS * c,
                    [list(ce.ap[0]), [NS, 2], [1, NS], [0, NS]])
            nc.vector.tensor_tensor(B1[:, c], xi, cj, OP.subtract)

        B4f = B4.rearrange("p c a b -> p (c a b)")

        # per-axis ScalarE chains interleaved with DVE combines so neither
        # engine stalls long at the head
        for c in range(3):
            d_c = B1[:, c].rearrange("p s a b -> p (s a b)")
            sq_c = B2[:, c].rearrange("p s a b -> p (s a b)")
            pp_c = B3[:, c].rearrange("p s a b -> p (s a b)")
            nc.scalar.activation(sq_c, d_c, AF.Square)                     # d^2
            nc.scalar.activation(pp_c, d_c, AF.Exp,
                                 bias=biasc[:, 0:1], scale=-s2aL)          # p+
            # jastrow banded differences jdx[c,dd,i] = x[c,i] - x[c,i+dd+1]
            # (per-axis DVE filler while ScalarE runs the slater chains)
            xib = AP(xe.tensor, xe.offset + 96 * c,
                     [ppart, [0, ND], [1, N]])
            xsk = AP(xe.tensor, xe.offset + 96 * c + 1,
                     [ppart, [1, ND], [1, N]])
            nc.vector.tensor_tensor(B4[:, c], xib, xsk, OP.subtract)
            nc.scalar.activation(d_c, d_c, AF.Exp,
                                 bias=biasc[:, 0:1], scale=s2aL)           # p- (in place)
            nc.scalar.activation(sq_c, sq_c, AF.Exp, scale=-alpha)         # e0 (in place)
            nc.vector.tensor_tensor(pp_c, pp_c, d_c, OP.add)               # q = p+ + p-
            # f = (q + 1) * e0, written bf16 so the factor products run 2x
            fb_c = fb[:, c].rearrange("p s a b -> p (s a b)")
            nc.vector.scalar_tensor_tensor(fb_c, pp_c, 1.0, sq_c,
                                           OP.add, OP.mult)
        nc.vector.tensor_tensor(tprod, fb[:, 0], fb[:, 1], OP.mult)
        nc.vector.tensor_tensor(Abft, tprod, fb[:, 2], OP.mult)

        # jastrow wrap chain on ScalarE (queued after slater ACT work):
        # u = |dx|; b = |u - L/2|; wr2 = (L/2 - b)^2   (into B1, in place)
        B1f = B1.rearrange("p c s a b -> p (c s a b)")
        nc.scalar.activation(B1f, B4f, AF.Abs)
        nc.scalar.activation(B1f, B1f, AF.Abs, bias=biasc[:, 1:2])
        nc.scalar.activation(B1f, B1f, AF.Square,
                             bias=biasc[:, 2:3], scale=-1.0)              # wrapped^2

        # =========================================================
        # Batched Gaussian elimination w/ partial pivoting (both spins)
        # column-major A; pivot search on squared candidates
        # =========================================================
        c2b = pool.tile([P, 2, NS], F32, tag="c2b")
        c2m = pool.tile([P, 2, NS], F32, tag="c2m")
        indb = pool.tile([P, 2, NS], F32, tag="indb")
        indbb = pool.tile([P, 2, NS], BF16, tag="indbb")
        Mbarch = pool.tile([P, 2, NS], F32, tag="Mbarch")
        prs = pool.tile([P, 2, NS - 1], F32, tag="prs")
        prsb = pool.tile([P, 2, NS - 1, 2], BF16, tag="prsb")   # pair-duplicated
        rpv = pool.tile([P, 2, 1], F32, tag="rpv")
        maskw = pool.tile([P, 2, NS], F32, tag="maskw")
        prowall = pool.tile([P, 2, NS, NS], F32, tag="prowall")
        scr = pool.tile([P, 2, NS, NS], F32, tag="scr")
        scrb = pool.tile([P, 2, NS, NS], BF16, tag="scrb")

        # jastrow intermediates
        r2 = pool.tile([P, ND, N], F32, tag="r2")
        jq = pool.tile([P, ND, N], F32, tag="jq")      # 1/r -> G -> P
        jr = pool.tile([P, ND, N], F32, tag="jr")      # relu -> lnw -> 1/w
        jdec = pool.tile([P, ND, N], F32, tag="jdec")  # decay
        jes = pool.tile([P, ND, N], F32, tag="jes")    # ln r2 -> rF -> e_sel
        jsum = pool.tile([P, 1], F32, tag="jsum")
        labs = pool.tile([P, 2, NS], F32, tag="labs")
        ld1 = pool.tile([P, 1], F32, tag="ld1")

        # maskw: tie-break weights; used rows go negative (-2 trick) and are
        # never picked again (candidates are squares, so >= 0)
        nc.default_dma_engine.dma_start(maskw, wcs[:])

        # views of B1 (wrapped^2) per axis, shaped like r2
        wr2 = [AP(B1.tensor, B1.offset + 2048 * c,
                  [list(B1.ap[0]), [N, ND], [1, N]])
               for c in range(3)]

        def search_ops(k):
            """Squared-candidate argmax for step k (indicator into indb)."""
            A = Abft if k < KBF else Abuf
            colk = A[:, :, k, :]
            nc.vector.tensor_tensor(c2b, colk, colk, OP.mult)
            nc.vector.tensor_tensor(c2m, c2b, maskw, OP.mult)
            nc.vector.reduce_max(Mbarch[:, :, k], c2m, axis=AX.X)
            if k < NS - 1:
                ind = indbb if k < KBF else indb
                nc.vector.tensor_tensor(
                    ind, c2m,
                    Mbarch[:, :, k:k + 1].broadcast_to([P, 2, NS]), OP.is_equal
                )
                nc.vector.scalar_tensor_tensor(
                    maskw, ind, -2.0, maskw, OP.mult, OP.add
                )

        def extract_ops(k):
            """Pivot row extraction for step k into prowall[:, :, k, :T]."""
            T = NS - k
            if k < KBF:
                nc.vector.tensor_tensor(
                    scrb[:, :, :T, :],
                    Abft[:, :, k:, :],
                    indbb[:, :, None, :].broadcast_to([P, 2, T, NS]),
                    OP.mult,
                )
                nc.vector.reduce_sum(prowall[:, :, k, :T], scrb[:, :, :T, :],
                                     axis=AX.X)
            else:
                nc.vector.tensor_tensor(
                    scr[:, :, :T, :],
                    Abuf[:, :, k:, :],
                    indb[:, :, None, :].broadcast_to([P, 2, T, NS]),
                    OP.mult,
                )
                nc.vector.reduce_sum(prowall[:, :, k, :T], scr[:, :, :T, :],
                                     axis=AX.X)

        search_ops(0)
        extract_ops(0)
        for k in range(NS - 1):
            T = NS - k
            # jastrow fillers at fixed steps (DVE r2 adds, ScalarE chains,
            # GpSimd tail) -- all hidden inside the GE window
            if k == 2:
                nc.vector.tensor_tensor(r2, wr2[0], wr2[1], OP.add)
            elif k == 4:
                nc.vector.tensor_tensor(r2, r2, wr2[2], OP.add)
            elif k == 5:
                # grouped by activation family to minimise ACT table swaps
                nc.scalar.activation(jr, r2, AF.Relu, bias=biasc[:, 5:6],
                                     scale=-0.01)
                nc.scalar.activation(jes, r2, AF.Ln)                        # ln r2
                nc.scalar.activation(jr, jr, AF.Ln, bias=biasc[:, 4:5])     # ln w
                nc.scalar.activation(jq, jes, AF.Exp, scale=-0.5)           # 1/r
                nc.scalar.activation(r2, jes, AF.Exp, scale=0.5)            # r
                nc.scalar.activation(jr, jr, AF.Exp, scale=-1.0)            # 1/w
                nc.scalar.activation(jdec, jr, AF.Exp, bias=1.0, scale=-1.0)  # decay
            elif k == 14:
                nc.vector.tensor_tensor(jes, r2, Finv, OP.mult)         # -r/F_sel
            elif k == 15:
                nc.scalar.activation(jes, jes, AF.Exp)                  # e_sel
            elif k == 16:
                # preload the Ln activation table so the final logdet pass
                # does not pay a table swap on the critical tail
                nc.scalar.activation(ld1, biasc[:, 3:4], AF.Ln)
            elif k == 20:
                # w*(e_sel - 1)
                nc.vector.scalar_tensor_tensor(
                    jes, jes, -1.0, wcol.broadcast_to([P, ND, N]),
                    OP.add, OP.mult)
            elif k == 21:
                nc.vector.tensor_tensor(jq, jq, jdec, OP.mult)          # G = q*decay
            elif k == 23:
                nc.vector.tensor_tensor(jq, jq, jes, OP.mult)           # w*G*(e_sel-1)
            elif k == 27:
                # jast = Aj * sum(w*G*(e_sel-1))
                nc.scalar.activation(B4[:, 0], jq, AF.Copy, scale=Aj,
                                     bias=float(WBIAS), accum_out=jsum)

            # scaled pivot row: prs[j] = prow[j+1] * (1/pivot)
            nc.vector.reciprocal(rpv, prowall[:, :, k, 0:1])
            if k < KBF:
                # prs written pair-duplicated so the outer product keeps a
                # packed [stride1, 2] innermost on every operand (2x bf16)
                nc.vector.tensor_tensor(
                    prsb[:, :, :T - 1, :],
                    prowall[:, :, k, 1:T, None].broadcast_to([P, 2, T - 1, 2]),
                    rpv[:, :, :, None].broadcast_to([P, 2, T - 1, 2]),
                    OP.mult,
                )
                for sp in range(2):
                    colk5 = AP(Abft.tensor,
                               Abft.offset + sp * NS * NS + k * NS,
                               [list(Abft.ap[0]), [0, T - 1],
                                [2, NS // 2], [1, 2]])
                    prs5 = AP(prsb.tensor, prsb.offset + sp * 2 * (NS - 1),
                              [list(prsb.ap[0]), [2, T - 1],
                               [0, NS // 2], [1, 2]])
                    scr5 = AP(scrb.tensor, scrb.offset + sp * NS * NS,
                              [list(scrb.ap[0]), [NS, T - 1],
                               [2, NS // 2], [1, 2]])
                    nc.vector.tensor_tensor(scr5, colk5, prs5, OP.mult)
                A, scrc = Abft, scrb
            else:
                A, scrc = Abuf, scr
                nc.vector.tensor_tensor(
                    prs[:, :, :T - 1],
                    prowall[:, :, k, 1:T],
                    rpv.broadcast_to([P, 2, T - 1]),
                    OP.mult,
                )
                colk = A[:, :, k, :]
                # outer product scr[j,i] = colk[i] * prs[j]
                nc.vector.tensor_tensor(
                    scrc[:, :, :T - 1, :],
                    colk[:, :, None, :].broadcast_to([P, 2, T - 1, NS]),
                    prs[:, :, :T - 1, None].broadcast_to([P, 2, T - 1, NS]),
                    OP.mult,
                )
            # one flat subtract over cols k+1.. ; at the precision transition
            # the result lands directly in the f32 matrix (no separate copy)
            Aout = Abuf if k + 1 == KBF else A
            nc.vector.tensor_tensor(
                Aout[:, :, k + 1:, :].rearrange("p s a b -> p s (a b)"),
                A[:, :, k + 1:, :].rearrange("p s a b -> p s (a b)"),
                scrc[:, :, :T - 1, :].rearrange("p s a b -> p s (a b)"),
                OP.subtract,
            )
            search_ops(k + 1)
            if k + 1 < NS - 1:
                extract_ops(k + 1)

        # =========================================================
        # logdet:  Mb_k = piv_k^2 * w_ipiv, and every row is pivoted exactly
        # once, so  ld = 0.5*(sum_k ln Mb_k - sum_i ln w_i)  with the weight
        # constant folded into jsum's accumulation bias.
        # =========================================================
        nc.scalar.activation(labs, Mbarch, AF.Ln, bias=biasc[:, 3:4])
        nc.vector.reduce_sum(ld1, labs.rearrange("p s a -> p (s a)"), axis=AX.X)
        ob = pool.tile([P, 1], F32, tag="ob")
        nc.vector.scalar_tensor_tensor(ob, ld1, 0.5, jsum, OP.mult, OP.add)
        nc.default_dma_engine.dma_start(outp[:], ob)

    nc.finalize()
    return nc


_CACHE = {}


def _get_built(alpha: float):
    key = round(alpha, 9)
    if key not in _CACHE:
        _CACHE[key] = _build(alpha)
    return _CACHE[key]


def _make_inputs(walkerRs: np.ndarray):
    cu, cd = _centers()
    cen = np.stack([cu, cd], 0)                   # (2, NS, 3)
    cst = np.ascontiguousarray(
        np.broadcast_to(cen.transpose(2, 0, 1)[None], (P, 3, 2, NS))
    ).astype(np.float32)
    w = (1.0 + np.arange(NS) * 2.0 ** -21).astype(np.float32)
    wcs = np.ascontiguousarray(
        np.broadcast_to(w[None, None, :], (P, 2, NS))
    ).astype(np.float32)
    in_maps = []
    for c in range(NCORES):
        sh = walkerRs[c * P:(c + 1) * P] % L      # (P, N, 3)
        xt = sh.transpose(0, 2, 1)                # (P, 3, N)
        x2 = np.concatenate([xt, xt[:, :, :NS]], axis=2)   # (P, 3, 96)
        in_maps.append({"xsh": np.ascontiguousarray(x2).astype(np.float32),
                        "cst": cst, "wcs": wcs})
    return in_maps


def kernel(walkerRs: np.ndarray, log_alpha: np.ndarray, _trace=False):
    walkerRs = np.asarray(walkerRs, dtype=np.float32)
    la = float(np.asarray(log_alpha))
    alpha = float(np.clip(np.exp(la), 55.0 / L ** 2, 300.0 / L ** 2))
    nc = _get_built(alpha)
    in_maps = _make_inputs(walkerRs)
    res = None
    for attempt in range(3):
        try:
            res = run_bass_kernel_spmd(nc, in_maps, list(range(NCORES)),
                                       trace=_trace)
            break
        except Exception:
            # transient NRT "device unrecoverable" after a prior bad run
            if attempt == 2:
                raise
            import time as _time
            _time.sleep(15)
    out = np.concatenate([res.results[i]["out"][:, 0] for i in range(NCORES)])
    if _trace:
        return out.astype(np.float32), res
    return out.astype(np.float32)
